# revision 11
# baseline (speedup 1.0000x reference)
"""Trainium2 Bass kernel for nn_MeshUpConv (3x chained SplineConv, deg-2 2D
B-spline, N=100k nodes, E=1.6M edges) on 8 NeuronCores.

Strategy (destination-bucketed graph parallel):
  - Host: bucket edges by destination-owner core; within a core group by
    (source-block-group, 256-dst-window). Per (core, group, window) the edge
    count is padded to whole 128-edge tiles with counts equalized across
    cores so the SPMD program is shared. Spline basis products (9 per edge)
    are precomputed on host and shipped as a bf16 slab; all node/weight
    tensors ship pre-cast to bf16.
  - Device, per layer:
      * node phase: every core computes the FULL node-level transform table
        xW [Npad, 640] (bf16 rows: 576 = [o-major, s-minor] + 64 zero pad)
        via PE matmuls into its own DRAM.
      * edge phase, per source-block group g: dma_gather (int16 block-local
        src ids) pulls xW rows in 12-tile batches; DVE multiplies by the 9
        spline basis products (broadcast AP, 2x packed mode) and reduces
        over the 9 slots with a packed add-tree (instead of the 1x-mode
        tensor_reduce); per-tile selection matrices built with
        tensor_scalar is_equal (4x mode); two accumulating matmuls per tile
        scatter-add into per-window PSUM, flushed into an SBUF accumulator
        slab. Root terms via per-window matmuls. Final: ReLU (+transpose
        for the next layer's feature-major consumption).
      * AllGather (partition-concat) of transposed shard outputs between
        layers.
"""
import sys

sys.path.insert(0, "/opt/trn_rl_repo")

from contextlib import ExitStack
from dataclasses import dataclass

import numpy as np
import ml_dtypes

import concourse.bass as bass
import concourse.tile as tile
from concourse import mybir
from concourse.masks import make_identity

F32 = mybir.dt.float32
BF16 = mybir.dt.bfloat16
I16 = mybir.dt.int16
AF = mybir.ActivationFunctionType
OP = mybir.AluOpType

S = 9
CO = 64
ROWW = S * CO  # 576 payload row width
ROWP = 640  # padded table row (1280 B, %256)
WIN = 128

BF = ml_dtypes.bfloat16


@dataclass
class Cfg:
    ncores: int = 8
    nsh: int = 12500  # real nodes per core
    nw: int = 98  # 128-dst windows per core
    cin: int = 128
    ch: int = 896  # node-chunk (divides shp, multiple of 128)
    ngrp: int = 4  # source block-pair groups
    gb: int = 12  # tiles per dma_gather batch (= DVE batch)

    @property
    def shp(self):
        return self.nw * WIN

    @property
    def npad(self):
        return self.ncores * self.shp

    @property
    def n(self):
        return self.ncores * self.nsh


FULL = Cfg()


def _bspline2(u):
    return np.stack(
        [0.5 * (1.0 - u) ** 2, -u * u + u + 0.5, 0.5 * u * u], axis=-1
    ).astype(np.float32)


# --------------------------------------------------------------------------
# host-side schedule / sharding
# --------------------------------------------------------------------------
def host_prep(cfg, x, skip, edge_index, edge_attr, W1, root1, W2, root2):
    ncores, nsh, nw, shp = cfg.ncores, cfg.nsh, cfg.nw, cfg.shp
    ngrp = cfg.ngrp
    bpg = ncores // ngrp  # blocks per group (2)
    nv = (nw + 1) // 2  # 256-dst windows (49)
    src = np.asarray(edge_index[0]).astype(np.int64)
    dst = np.asarray(edge_index[1]).astype(np.int64)
    attr = np.asarray(edge_attr, dtype=np.float32)
    # spline basis products on host: beta[e, s] with s = k0 + 3*k1
    b0 = _bspline2(attr[:, 0])  # [E,3]
    b1 = _bspline2(attr[:, 1])  # [E,3]
    beta = (b1[:, :, None] * b0[:, None, :]).reshape(-1, S)  # [E,9]

    owner = dst // nsh
    dloc = dst - owner * nsh
    sblk = src // nsh
    grp = sblk // bpg
    sloc = (sblk % bpg) * shp + (src - sblk * nsh)
    vwin = dloc // 256

    cnt = np.zeros((ncores, ngrp, nv), np.int64)
    for m in range(ncores):
        for g in range(ngrp):
            sel = (owner == m) & (grp == g)
            cnt[m, g] = np.bincount(vwin[sel], minlength=nv)
    tcv = np.maximum(1, -(-cnt.max(axis=0) // WIN)).astype(np.int64)  # [ngrp, nv]
    gtc = tcv.sum(axis=1)
    goff = np.concatenate([[0], np.cumsum(gtc)]).astype(np.int64)
    ttot = int(goff[-1])
    ne = ttot * WIN
    voff = np.zeros((ngrp, nv + 1), np.int64)
    for g in range(ngrp):
        voff[g] = np.concatenate([[goff[g]], goff[g] + np.cumsum(tcv[g])])

    # pad slots carry idx -1: all padding is at bucket tails, and SWDGE
    # desc-gen trims trailing negative indices (no descriptor, no transfer)
    srcs = np.full((ncores, ne), -1, np.int64)
    dst_rel = np.zeros((ncores, ne), np.int16)
    bets = np.zeros((ncores, ne, S), np.float32)
    for m in range(ncores):
        own = owner == m
        for g in range(ngrp):
            ing = own & (grp == g)
            for v in range(nv):
                sel = np.where(ing & (vwin == v))[0]
                k = len(sel)
                base = int(voff[g, v]) * WIN
                srcs[m, base : base + k] = sloc[sel]
                dst_rel[m, base : base + k] = (dloc[sel] - v * 256).astype(np.int16)
                bets[m, base : base + k] = beta[sel]

    def tilize(a):
        a = a.reshape(ttot, WIN, *a.shape[1:])
        return np.ascontiguousarray(np.swapaxes(a, 0, 1))

    def idx_wrap(a):
        w = a.reshape(-1, 16).T.astype(np.int16)
        return np.ascontiguousarray(np.tile(w, (8, 1)))

    srcs_w = np.stack([idx_wrap(srcs[m]) for m in range(ncores)])
    dstr_t = np.stack(
        [tilize(dst_rel[m]).astype(np.float32) for m in range(ncores)]
    )
    beta_t = np.stack(
        [tilize(bets[m]).reshape(WIN, ttot * S).astype(BF) for m in range(ncores)]
    )

    cin = cfg.cin
    xpad = np.zeros((cfg.npad, cin), np.float32)
    spad = np.zeros((cfg.npad, CO), np.float32)
    for m in range(ncores):
        xpad[m * shp : m * shp + nsh] = x[m * nsh : (m + 1) * nsh]
        spad[m * shp : m * shp + nsh] = skip[m * nsh : (m + 1) * nsh]
    xT = np.ascontiguousarray(xpad.T).astype(BF)
    skipT = np.ascontiguousarray(spad.T).astype(BF)

    def wall(W, fdim):
        w = np.transpose(np.asarray(W, np.float32), (1, 2, 0)).reshape(fdim, ROWW)
        return np.ascontiguousarray(
            np.concatenate([w, np.zeros((fdim, ROWP - ROWW), np.float32)], axis=1)
        ).astype(BF)

    W1all = wall(W1, cin)
    W2all = wall(W2, CO)
    iota256 = np.ascontiguousarray(
        np.tile(np.arange(256, dtype=np.int16), (WIN, 1))
    )

    shared = dict(
        xT=xT,
        skipT=skipT,
        W1all=W1all,
        W2all=W2all,
        root1=np.asarray(root1, np.float32).astype(BF),
        root2=np.asarray(root2, np.float32).astype(BF),
        iota=iota256,
    )
    in_maps = []
    for m in range(ncores):
        d = dict(shared)
        d["xTown"] = np.ascontiguousarray(xT[:, m * shp : (m + 1) * shp])
        d["skipTown"] = np.ascontiguousarray(skipT[:, m * shp : (m + 1) * shp])
        d["srcs"] = srcs_w[m]
        d["dstr"] = dstr_t[m]
        d["betas"] = beta_t[m]
        in_maps.append(d)
    sched = dict(
        ttot=ttot,
        gbmax=int(tcv.max()),
        tcv=[[int(v) for v in row] for row in tcv],
        voff=[[int(v) for v in row] for row in voff],
    )
    return in_maps, sched


# --------------------------------------------------------------------------
# device program
# --------------------------------------------------------------------------
def build_program(cfg, sched):
    from concourse import bacc

    nc = bacc.Bacc(
        "TRN2", target_bir_lowering=False, debug=False, num_devices=cfg.ncores
    )
    cin, shp, npad, nw = cfg.cin, cfg.shp, cfg.npad, cfg.nw
    ch = cfg.ch
    ntch = ch // WIN
    assert shp % ch == 0
    nchunks = shp // ch
    ttot = sched["ttot"]
    gbmax = sched["gbmax"]
    tcv = sched["tcv"]
    voff = sched["voff"]
    ngrp = cfg.ngrp
    grows = (cfg.ncores // ngrp) * shp
    nv = (nw + 1) // 2

    xT = nc.declare_dram_parameter("xT", [cin, npad], BF16, isOutput=False)
    xTown = nc.declare_dram_parameter("xTown", [cin, shp], BF16, isOutput=False)
    skipT = nc.declare_dram_parameter("skipT", [CO, npad], BF16, isOutput=False)
    skipTown = nc.declare_dram_parameter("skipTown", [CO, shp], BF16, isOutput=False)
    W1all_d = nc.declare_dram_parameter("W1all", [cin, ROWP], BF16, isOutput=False)
    W2all_d = nc.declare_dram_parameter("W2all", [CO, ROWP], BF16, isOutput=False)
    root1_d = nc.declare_dram_parameter("root1", [cin, CO], BF16, isOutput=False)
    root2_d = nc.declare_dram_parameter("root2", [CO, CO], BF16, isOutput=False)
    iota_d = nc.declare_dram_parameter("iota", [WIN, 256], I16, isOutput=False)
    srcs_d = nc.declare_dram_parameter(
        "srcs", [WIN, (ttot * WIN) // 16], I16, isOutput=False
    )
    dstr_d = nc.declare_dram_parameter("dstr", [WIN, ttot], F32, isOutput=False)
    betas_d = nc.declare_dram_parameter(
        "betas", [WIN, ttot * S], BF16, isOutput=False
    )
    out_d = nc.declare_dram_parameter("out_shard", [shp, CO], F32, isOutput=True)

    tables = [nc.dram_tensor(f"table{i}", [npad, ROWP], BF16) for i in range(3)]
    bounceT = [nc.dram_tensor(f"bounceT{i}", [CO, shp], BF16) for i in range(2)]
    hstackT = [
        nc.dram_tensor(
            f"hstackT{i}", [cfg.ncores * CO, shp], BF16, addr_space="Shared"
        )
        for i in range(2)
    ]

    with tile.TileContext(nc) as tc:
        with ExitStack() as ctx:
            persist = ctx.enter_context(tc.tile_pool(name="persist", bufs=1))
            work = ctx.enter_context(tc.tile_pool(name="work", bufs=2))
            gpool = ctx.enter_context(tc.tile_pool(name="gpool", bufs=2))
            dpool = ctx.enter_context(tc.tile_pool(name="dpool", bufs=2))
            spool = ctx.enter_context(tc.tile_pool(name="spool", bufs=3))
            psum = ctx.enter_context(tc.tile_pool(name="psum", bufs=2, space="PSUM"))
            psum1 = ctx.enter_context(tc.tile_pool(name="psum1", bufs=1, space="PSUM"))
            psumw = ctx.enter_context(tc.tile_pool(name="psumw", bufs=1, space="PSUM"))

            # ------------- prep -------------
            iota_t = persist.tile([WIN, 256], I16, tag="iota")
            nc.sync.dma_start(iota_t[:], iota_d[:, :])
            ident = persist.tile([WIN, WIN], BF16, tag="ident")
            make_identity(nc, ident[:])

            def load_bf(dram_ap, shape, tag):
                out = persist.tile(shape, BF16, tag=tag)
                nc.sync.dma_start(out[:], dram_ap)
                return out

            W1b = load_bf(W1all_d[:, :], [cin, ROWP], "W1b")
            W1bh = load_bf(W1all_d[CO:cin, :], [CO, ROWP], "W1bh")
            W2b = load_bf(W2all_d[:, :], [CO, ROWP], "W2b")
            root1b = load_bf(root1_d[:, :], [cin, CO], "root1b")
            root1bh = load_bf(root1_d[CO:cin, :], [CO, CO], "root1bh")
            root2b = load_bf(root2_d[:, :], [CO, CO], "root2b")

            dst_sl = persist.tile([WIN, ttot], F32, tag="dst")
            nc.sync.dma_start(dst_sl[:], dstr_d[:, :])
            src_sl = persist.tile([WIN, ttot * 8], I16, tag="src16")
            nc.sync.dma_start(src_sl[:], srcs_d[:, :])
            betas = persist.tile([WIN, ttot * S], BF16, tag="betas")
            nc.sync.dma_start(betas[:], betas_d[:, :])
            # window accumulator slab [128, nw*64] f32
            hacc = persist.tile([WIN, nw * CO], F32, tag="hacc")

            # zero the rotating gather slabs once: slots belonging to trimmed
            # (idx -1) pad rows are never written by the gather, and stale
            # uninitialized SBUF could hold Inf/NaN bit patterns that the
            # beta=0 multiply would turn into NaN (0 * Inf).
            for _ in range(2):
                gz = gpool.tile([WIN, gbmax * ROWP], BF16, tag="g")
                nc.vector.memset(gz[:], 0.0)

            tc.strict_bb_all_engine_barrier()

            # ------------- layers -------------
            def table_phase(li):
                table = tables[li]
                for blk in range(cfg.ncores):
                    for cc in range(nchunks):
                        n0 = blk * shp + cc * ch
                        if li == 0:
                            xc = work.tile([cin, ch], BF16, tag="rcb")
                            nc.sync.dma_start(xc[:], xT[:, n0 : n0 + ch])
                            halves = [(xc, W1b, None)]
                        elif li == 1:
                            hc = work.tile([CO, ch], BF16, tag="rcb2")
                            nc.sync.dma_start(
                                hc[:],
                                hstackT[0][
                                    blk * CO : (blk + 1) * CO, cc * ch : (cc + 1) * ch
                                ],
                            )
                            sc = work.tile([CO, ch], BF16, tag="scb")
                            nc.sync.dma_start(sc[:], skipT[:, n0 : n0 + ch])
                            halves = [(hc, W1b, slice(0, CO)), (sc, W1bh, None)]
                        else:
                            hc = work.tile([CO, ch], BF16, tag="rcb2")
                            nc.sync.dma_start(
                                hc[:],
                                hstackT[1][
                                    blk * CO : (blk + 1) * CO, cc * ch : (cc + 1) * ch
                                ],
                            )
                            halves = [(hc, W2b, None)]
                        for ti in range(ntch):
                            ps = psum.tile([WIN, ROWP], F32, tag="pstab")
                            sl = slice(ti * WIN, (ti + 1) * WIN)
                            for c0, c1 in ((0, 512), (512, ROWP)):
                                for hi, (lh, wb, rows) in enumerate(halves):
                                    rhs = (
                                        wb[rows, c0:c1]
                                        if rows is not None
                                        else wb[:, c0:c1]
                                    )
                                    nc.tensor.matmul(
                                        ps[:, c0:c1],
                                        lhsT=lh[:, sl],
                                        rhs=rhs,
                                        start=(hi == 0),
                                        stop=(hi == len(halves) - 1),
                                    )
                            tb = work.tile([WIN, ROWP], BF16, tag="tbb")
                            nc.scalar.activation(tb[:], ps[:], AF.Copy)
                            nc.sync.dma_start(
                                table[n0 + ti * WIN : n0 + (ti + 1) * WIN, :], tb[:]
                            )

            def edge_phase(li):
                table = tables[li]
                rootb = root1b if li < 2 else root2b
                rw = ch // WIN
                GBM = gbmax
                nc.vector.memset(hacc[:], 0.0)
                for g in range(ngrp):
                    for v in range(nv):
                        ntv = int(tcv[g][v])
                        t0 = int(voff[g][v])
                        bt = ntv
                        # bucket-aligned gather: trailing pad slots carry idx
                        # -1 and are trimmed by SWDGE desc-gen (no descriptor,
                        # no transfer); their stale SBUF slots are zeroed by
                        # the beta=0 multiply downstream.
                        g_tile = gpool.tile([WIN, GBM * ROWP], BF16, tag="g")
                        nc.gpsimd.dma_gather(
                            out_ap=g_tile[:, : bt * ROWP].rearrange(
                                "p (t c) -> p t c", c=ROWP
                            ),
                            in_ap=table[g * grows : (g + 1) * grows, :],
                            idxs_ap=src_sl[:, t0 * 8 : (t0 + bt) * 8],
                            num_idxs=bt * WIN,
                            num_idxs_reg=bt * WIN,
                            elem_size=ROWP,
                            single_packet=False,
                        )
                        # beta-apply (2x packed) + s-reduce add-tree
                        qv = (
                            g_tile[:, : bt * ROWP]
                            .rearrange("p (t c) -> p t c", c=ROWP)[:, :, :ROWW]
                            .rearrange("p t (o s) -> p t o s", o=CO, s=S)
                        )
                        bv = (
                            betas[:, t0 * S : (t0 + bt) * S]
                            .rearrange("p (t s) -> p t s", s=S)
                            .rearrange("p t (x s) -> p t x s", x=1)
                            .to_broadcast([WIN, bt, CO, S])
                        )
                        q = dpool.tile([WIN, GBM * ROWW], BF16, tag="q")
                        qq = q[:, : bt * ROWW].rearrange(
                            "p (t o s) -> p t o s", o=CO, s=S
                        )
                        nc.vector.tensor_tensor(qq, qv, bv, op=OP.mult)
                        r1 = dpool.tile([WIN, GBM * CO * 4], BF16, tag="r1")
                        r1v = r1[:, : bt * CO * 4].rearrange(
                            "p (t o k) -> p t o k", o=CO, k=4
                        )
                        nc.vector.tensor_tensor(
                            r1v, qq[:, :, :, 0:4], qq[:, :, :, 4:8], op=OP.add
                        )
                        r2 = dpool.tile([WIN, GBM * CO * 2], BF16, tag="r2")
                        r2v = r2[:, : bt * CO * 2].rearrange(
                            "p (t o k) -> p t o k", o=CO, k=2
                        )
                        nc.vector.tensor_tensor(
                            r2v, r1v[:, :, :, 0:2], r1v[:, :, :, 2:4], op=OP.add
                        )
                        d_acc = dpool.tile([WIN, GBM * CO], BF16, tag="dacc")
                        dav = d_acc[:, : bt * CO].rearrange(
                            "p (t o) -> p t o", o=CO
                        )
                        nc.vector.tensor_tensor(
                            dav, r2v[:, :, :, 0], r2v[:, :, :, 1], op=OP.add
                        )
                        nc.vector.tensor_tensor(
                            dav, dav, qq[:, :, :, 8], op=OP.add
                        )
                        psw = psumw.tile([WIN, 576], F32, tag="psw")
                        for j in range(ntv):
                            t = t0 + j
                            d_sel = spool.tile([WIN, 256], BF16, tag="sel")
                            nc.vector.tensor_scalar(
                                out=d_sel[:],
                                in0=iota_t[:],
                                scalar1=dst_sl[:, t : t + 1],
                                scalar2=None,
                                op0=OP.is_equal,
                            )
                            nc.tensor.matmul(
                                psw[:, 0:CO],
                                lhsT=d_sel[:, 0:WIN],
                                rhs=d_acc[:, j * CO : (j + 1) * CO],
                                start=(j == 0),
                                stop=(j == ntv - 1),
                            )
                            nc.tensor.matmul(
                                psw[:, 512 : 512 + CO],
                                lhsT=d_sel[:, WIN:256],
                                rhs=d_acc[:, j * CO : (j + 1) * CO],
                                start=(j == 0),
                                stop=(j == ntv - 1),
                            )
                        # flush both sub-windows into hacc
                        for half in range(2):
                            w = 2 * v + half
                            if w >= nw:
                                continue
                            nc.vector.tensor_tensor(
                                hacc[:, w * CO : (w + 1) * CO],
                                hacc[:, w * CO : (w + 1) * CO],
                                psw[:, half * 512 : half * 512 + CO],
                                op=OP.add,
                            )
                # root pass: hacc[w] += h_own @ root
                rootc = None
                skownc = None
                rootc_cc = -1
                for w in range(nw):
                    cc = w // rw
                    if cc != rootc_cc:
                        rootc_cc = cc
                        if li == 0:
                            rootc = work.tile([cin, ch], BF16, tag="rcb")
                            nc.sync.dma_start(
                                rootc[:], xTown[:, cc * ch : (cc + 1) * ch]
                            )
                        else:
                            rootc = work.tile([CO, ch], BF16, tag="rcb2")
                            nc.sync.dma_start(
                                rootc[:],
                                bounceT[li - 1][:, cc * ch : (cc + 1) * ch],
                            )
                            if li == 1:
                                skownc = work.tile([CO, ch], BF16, tag="rcb")
                                nc.sync.dma_start(
                                    skownc[:], skipTown[:, cc * ch : (cc + 1) * ch]
                                )
                    wsl = slice((w % rw) * WIN, (w % rw + 1) * WIN)
                    psr = psum1.tile([WIN, CO], F32, tag="ps2")
                    if li == 1:
                        nc.tensor.matmul(
                            psr[:], lhsT=rootc[:, wsl], rhs=rootb[0:CO, :],
                            start=True, stop=False,
                        )
                        nc.tensor.matmul(
                            psr[:], lhsT=skownc[:, wsl], rhs=root1bh[:, :],
                            start=False, stop=True,
                        )
                    else:
                        nc.tensor.matmul(
                            psr[:], lhsT=rootc[:, wsl], rhs=rootb[:, :],
                            start=True, stop=True,
                        )
                    nc.vector.tensor_tensor(
                        hacc[:, w * CO : (w + 1) * CO],
                        hacc[:, w * CO : (w + 1) * CO],
                        psr[:],
                        op=OP.add,
                    )
                # final: relu + out/transpose per window
                for w in range(nw):
                    hv = hacc[:, w * CO : (w + 1) * CO]
                    if li < 2:
                        hb = work.tile([WIN, CO], BF16, tag="hb")
                        nc.scalar.activation(hb[:], hv, AF.Relu)
                        pst = psum1.tile([CO, WIN], BF16, tag="ps2")
                        nc.tensor.transpose(out=pst[:], in_=hb[:], identity=ident[:])
                        hbT = work.tile([CO, WIN], BF16, tag="hbT")
                        nc.scalar.activation(hbT[:], pst[:], AF.Copy)
                        nc.sync.dma_start(
                            bounceT[li][:, w * WIN : (w + 1) * WIN], hbT[:]
                        )
                    else:
                        ho = work.tile([WIN, CO], F32, tag="ho")
                        nc.scalar.activation(ho[:], hv, AF.Relu)
                        nc.sync.dma_start(
                            out_d[w * WIN : (w + 1) * WIN, :], ho[:]
                        )

            for li in range(3):
                table_phase(li)
                tc.strict_bb_all_engine_barrier()
                edge_phase(li)
                tc.strict_bb_all_engine_barrier()
                if li < 2:
                    nc.gpsimd.collective_compute(
                        "AllGather",
                        OP.bypass,
                        replica_groups=[list(range(cfg.ncores))],
                        ins=[bounceT[li].ap().opt()],
                        outs=[hstackT[li].ap().opt()],
                    )
                    tc.strict_bb_all_engine_barrier()
    nc.finalize()
    return nc


# --------------------------------------------------------------------------
# entry point
# --------------------------------------------------------------------------
def run_full(inputs, trace=False, trace_kwargs=None):
    cfg = FULL
    in_maps, sched = host_prep(
        cfg,
        np.asarray(inputs["x"], np.float32),
        np.asarray(inputs["skip"], np.float32),
        inputs["edge_index"],
        np.asarray(inputs["edge_attr"], np.float32),
        inputs["W1"],
        inputs["root1"],
        inputs["W2"],
        inputs["root2"],
    )
    nc = build_program(cfg, sched)
    from concourse.bass_utils import run_bass_kernel_spmd

    res = run_bass_kernel_spmd(
        nc,
        in_maps,
        core_ids=list(range(cfg.ncores)),
        trace=trace,
        **(dict(trace_kwargs=trace_kwargs) if trace_kwargs else {}),
    )
    out = np.zeros((cfg.n, CO), np.float32)
    for m in range(cfg.ncores):
        shard = res.results[m]["out_shard"]
        out[m * cfg.nsh : (m + 1) * cfg.nsh] = shard[: cfg.nsh]
    return out, res


def kernel(**inputs):
    out, _ = run_full(inputs)
    return out


# revision 15
# speedup vs baseline: 1.2610x; 1.2610x over previous
"""Trainium2 Bass kernel for nn_MeshUpConv (3x chained SplineConv, deg-2 2D
B-spline, N=100k nodes, E=1.6M edges) on 8 NeuronCores.

Strategy (destination-bucketed graph parallel):
  - Host: bucket edges by destination-owner core; within a core group by
    (source-block-group, 256-dst-window). Per (core, group, window) the edge
    count is padded to whole 128-edge tiles with counts equalized across
    cores so the SPMD program is shared. Spline basis products (9 per edge)
    are precomputed on host and shipped as a bf16 slab; all node/weight
    tensors ship pre-cast to bf16.
  - Device, per layer:
      * node phase: every core computes the FULL node-level transform table
        xW [Npad, 640] (bf16 rows: 576 = [o-major, s-minor] + 64 zero pad)
        via PE matmuls into its own DRAM.
      * edge phase, per source-block group g: dma_gather (int16 block-local
        src ids) pulls xW rows in 12-tile batches; DVE multiplies by the 9
        spline basis products (broadcast AP, 2x packed mode) and reduces
        over the 9 slots with a packed add-tree (instead of the 1x-mode
        tensor_reduce); per-tile selection matrices built with
        tensor_scalar is_equal (4x mode); two accumulating matmuls per tile
        scatter-add into per-window PSUM, flushed into an SBUF accumulator
        slab. Root terms via per-window matmuls. Final: ReLU (+transpose
        for the next layer's feature-major consumption).
      * AllGather (partition-concat) of transposed shard outputs between
        layers.
"""
import sys

sys.path.insert(0, "/opt/trn_rl_repo")

from contextlib import ExitStack
from dataclasses import dataclass

import numpy as np
import ml_dtypes

import concourse.bass as bass
import concourse.tile as tile
from concourse import mybir
from concourse.masks import make_identity

F32 = mybir.dt.float32
BF16 = mybir.dt.bfloat16
I16 = mybir.dt.int16
AF = mybir.ActivationFunctionType
OP = mybir.AluOpType

S = 9
CO = 64
ROWW = S * CO  # 576 payload row width
ROWP = 640  # padded table row (1280 B, %256)
WIN = 128

BF = ml_dtypes.bfloat16


@dataclass
class Cfg:
    ncores: int = 8
    nsh: int = 12500  # real nodes per core
    nw: int = 98  # 128-dst windows per core
    cin: int = 128
    ch: int = 896  # node-chunk (divides shp, multiple of 128)
    ngrp: int = 4  # source block-pair groups
    gb: int = 12  # tiles per dma_gather batch (= DVE batch)

    @property
    def shp(self):
        return self.nw * WIN

    @property
    def npad(self):
        return self.ncores * self.shp

    @property
    def n(self):
        return self.ncores * self.nsh


FULL = Cfg()


def _bspline2(u):
    return np.stack(
        [0.5 * (1.0 - u) ** 2, -u * u + u + 0.5, 0.5 * u * u], axis=-1
    ).astype(np.float32)


# --------------------------------------------------------------------------
# host-side schedule / sharding
# --------------------------------------------------------------------------
def host_prep(cfg, x, skip, edge_index, edge_attr, W1, root1, W2, root2):
    ncores, nsh, nw, shp = cfg.ncores, cfg.nsh, cfg.nw, cfg.shp
    ngrp = cfg.ngrp
    bpg = ncores // ngrp  # blocks per group (2)
    nv = (nw + 1) // 2  # 256-dst windows (49)
    src = np.asarray(edge_index[0]).astype(np.int64)
    dst = np.asarray(edge_index[1]).astype(np.int64)
    attr = np.asarray(edge_attr, dtype=np.float32)
    # spline basis products on host: beta[e, s] with s = k0 + 3*k1
    b0 = _bspline2(attr[:, 0])  # [E,3]
    b1 = _bspline2(attr[:, 1])  # [E,3]
    beta = (b1[:, :, None] * b0[:, None, :]).reshape(-1, S)  # [E,9]

    owner = dst // nsh
    dloc = dst - owner * nsh
    sblk = src // nsh
    grp = sblk // bpg
    sloc = (sblk % bpg) * shp + (src - sblk * nsh)
    vwin = dloc // 256

    cnt = np.zeros((ncores, ngrp, nv), np.int64)
    for m in range(ncores):
        for g in range(ngrp):
            sel = (owner == m) & (grp == g)
            cnt[m, g] = np.bincount(vwin[sel], minlength=nv)
    tcv = np.maximum(1, -(-cnt.max(axis=0) // WIN)).astype(np.int64)  # [ngrp, nv]
    gtc = tcv.sum(axis=1)
    goff = np.concatenate([[0], np.cumsum(gtc)]).astype(np.int64)
    ttot = int(goff[-1])
    ne = ttot * WIN
    voff = np.zeros((ngrp, nv + 1), np.int64)
    for g in range(ngrp):
        voff[g] = np.concatenate([[goff[g]], goff[g] + np.cumsum(tcv[g])])

    zero_local = nsh  # block-local pad row (zero); lives in first block of grp
    srcs = np.full((ncores, ne), zero_local, np.int64)
    dst_rel = np.zeros((ncores, ne), np.int16)
    bets = np.zeros((ncores, ne, S), np.float32)
    for m in range(ncores):
        own = owner == m
        for g in range(ngrp):
            ing = own & (grp == g)
            for v in range(nv):
                sel = np.where(ing & (vwin == v))[0]
                k = len(sel)
                base = int(voff[g, v]) * WIN
                srcs[m, base : base + k] = sloc[sel]
                dst_rel[m, base : base + k] = (dloc[sel] - v * 256).astype(np.int16)
                bets[m, base : base + k] = beta[sel]

    def tilize(a):
        a = a.reshape(ttot, WIN, *a.shape[1:])
        return np.ascontiguousarray(np.swapaxes(a, 0, 1))

    def idx_wrap(a):
        w = a.reshape(-1, 16).T.astype(np.int16)
        return np.ascontiguousarray(np.tile(w, (8, 1)))

    srcs_w = np.stack([idx_wrap(srcs[m]) for m in range(ncores)])
    # dst ids and betas shipped as duplicated pairs so device-side broadcasts
    # ride an innermost packed pair (2x DVE mode)
    dstr_t = np.stack(
        [
            tilize(np.repeat(dst_rel[m], 2).reshape(ne, 2)).reshape(
                WIN, ttot * 2
            )
            for m in range(ncores)
        ]
    )
    beta_t = np.stack(
        [
            tilize(np.repeat(bets[m], 2, axis=-1))
            .reshape(WIN, ttot * 2 * S)
            .astype(BF)
            for m in range(ncores)
        ]
    )

    cin = cfg.cin
    xpad = np.zeros((cfg.npad, cin), np.float32)
    spad = np.zeros((cfg.npad, CO), np.float32)
    for m in range(ncores):
        xpad[m * shp : m * shp + nsh] = x[m * nsh : (m + 1) * nsh]
        spad[m * shp : m * shp + nsh] = skip[m * nsh : (m + 1) * nsh]
    xT = np.ascontiguousarray(xpad.T).astype(BF)
    skipT = np.ascontiguousarray(spad.T).astype(BF)

    def wall(W, fdim):
        # s-major row layout: row = [s=0..8][o=0..63]
        w = np.transpose(np.asarray(W, np.float32), (1, 0, 2)).reshape(fdim, ROWW)
        return np.ascontiguousarray(
            np.concatenate([w, np.zeros((fdim, ROWP - ROWW), np.float32)], axis=1)
        ).astype(BF)

    W1all = wall(W1, cin)
    W2all = wall(W2, CO)
    iota256 = np.ascontiguousarray(
        np.tile(np.arange(256, dtype=np.int16), (WIN, 1))
    )

    shared = dict(
        xT=xT,
        skipT=skipT,
        W1all=W1all,
        W2all=W2all,
        root1=np.asarray(root1, np.float32).astype(BF),
        root2=np.asarray(root2, np.float32).astype(BF),
        iota=iota256,
    )
    in_maps = []
    for m in range(ncores):
        d = dict(shared)
        d["xTown"] = np.ascontiguousarray(xT[:, m * shp : (m + 1) * shp])
        d["skipTown"] = np.ascontiguousarray(skipT[:, m * shp : (m + 1) * shp])
        d["srcs"] = srcs_w[m]
        d["dstr"] = dstr_t[m]
        d["betas"] = beta_t[m]
        in_maps.append(d)
    sched = dict(
        ttot=ttot,
        gbmax=int(tcv.max()),
        tcv=[[int(v) for v in row] for row in tcv],
        voff=[[int(v) for v in row] for row in voff],
    )
    return in_maps, sched


# --------------------------------------------------------------------------
# device program
# --------------------------------------------------------------------------
def build_program(cfg, sched):
    from concourse import bacc

    nc = bacc.Bacc(
        "TRN2", target_bir_lowering=False, debug=False, num_devices=cfg.ncores
    )
    cin, shp, npad, nw = cfg.cin, cfg.shp, cfg.npad, cfg.nw
    ch = cfg.ch
    ntch = ch // WIN
    assert shp % ch == 0
    nchunks = shp // ch
    ttot = sched["ttot"]
    gbmax = sched["gbmax"]
    tcv = sched["tcv"]
    voff = sched["voff"]
    ngrp = cfg.ngrp
    grows = (cfg.ncores // ngrp) * shp
    nv = (nw + 1) // 2

    xT = nc.declare_dram_parameter("xT", [cin, npad], BF16, isOutput=False)
    xTown = nc.declare_dram_parameter("xTown", [cin, shp], BF16, isOutput=False)
    skipT = nc.declare_dram_parameter("skipT", [CO, npad], BF16, isOutput=False)
    skipTown = nc.declare_dram_parameter("skipTown", [CO, shp], BF16, isOutput=False)
    W1all_d = nc.declare_dram_parameter("W1all", [cin, ROWP], BF16, isOutput=False)
    W2all_d = nc.declare_dram_parameter("W2all", [CO, ROWP], BF16, isOutput=False)
    root1_d = nc.declare_dram_parameter("root1", [cin, CO], BF16, isOutput=False)
    root2_d = nc.declare_dram_parameter("root2", [CO, CO], BF16, isOutput=False)
    iota_d = nc.declare_dram_parameter("iota", [WIN, 256], I16, isOutput=False)
    srcs_d = nc.declare_dram_parameter(
        "srcs", [WIN, (ttot * WIN) // 16], I16, isOutput=False
    )
    dstr_d = nc.declare_dram_parameter("dstr", [WIN, ttot * 2], I16, isOutput=False)
    betas_d = nc.declare_dram_parameter(
        "betas", [WIN, ttot * 2 * S], BF16, isOutput=False
    )
    out_d = nc.declare_dram_parameter("out_shard", [shp, CO], F32, isOutput=True)

    tables = [nc.dram_tensor(f"table{i}", [npad, ROWP], BF16) for i in range(3)]
    bounceT = [nc.dram_tensor(f"bounceT{i}", [CO, shp], BF16) for i in range(2)]
    hstackT = [
        nc.dram_tensor(
            f"hstackT{i}", [cfg.ncores * CO, shp], BF16, addr_space="Shared"
        )
        for i in range(2)
    ]

    with tile.TileContext(nc) as tc:
        with ExitStack() as ctx:
            persist = ctx.enter_context(tc.tile_pool(name="persist", bufs=1))
            work = ctx.enter_context(tc.tile_pool(name="work", bufs=2))
            gpool = ctx.enter_context(tc.tile_pool(name="gpool", bufs=2))
            dpool = ctx.enter_context(tc.tile_pool(name="dpool", bufs=2))
            spool = ctx.enter_context(tc.tile_pool(name="spool", bufs=3))
            psum = ctx.enter_context(tc.tile_pool(name="psum", bufs=2, space="PSUM"))
            psum1 = ctx.enter_context(tc.tile_pool(name="psum1", bufs=1, space="PSUM"))
            psumw = ctx.enter_context(tc.tile_pool(name="psumw", bufs=1, space="PSUM"))

            # ------------- prep -------------
            iota_t = persist.tile([WIN, 256], I16, tag="iota")
            nc.sync.dma_start(iota_t[:], iota_d[:, :])
            ident = persist.tile([WIN, WIN], BF16, tag="ident")
            make_identity(nc, ident[:])

            def load_bf(dram_ap, shape, tag):
                out = persist.tile(shape, BF16, tag=tag)
                nc.sync.dma_start(out[:], dram_ap)
                return out

            W1b = load_bf(W1all_d[:, :], [cin, ROWP], "W1b")
            W1bh = load_bf(W1all_d[CO:cin, :], [CO, ROWP], "W1bh")
            W2b = load_bf(W2all_d[:, :], [CO, ROWP], "W2b")
            root1b = load_bf(root1_d[:, :], [cin, CO], "root1b")
            root1bh = load_bf(root1_d[CO:cin, :], [CO, CO], "root1bh")
            root2b = load_bf(root2_d[:, :], [CO, CO], "root2b")

            dst_sl = persist.tile([WIN, ttot * 2], I16, tag="dst")
            nc.sync.dma_start(dst_sl[:], dstr_d[:, :])
            src_sl = persist.tile([WIN, ttot * 8], I16, tag="src16")
            nc.sync.dma_start(src_sl[:], srcs_d[:, :])
            betas = persist.tile([WIN, ttot * 2 * S], BF16, tag="betas")
            nc.sync.dma_start(betas[:], betas_d[:, :])

            # zero the rotating gather slabs once: slots belonging to trimmed
            # (idx -1) pad rows are never written by the gather, and stale
            # uninitialized SBUF could hold Inf/NaN bit patterns that the
            # beta=0 multiply would turn into NaN (0 * Inf).
            for _ in range(2):
                gz = gpool.tile([WIN, gbmax * ROWP], BF16, tag="g")
                nc.vector.memset(gz[:], 0.0)

            tc.strict_bb_all_engine_barrier()

            # ------------- layers -------------
            def table_phase(li):
                table = tables[li]
                for blk in range(cfg.ncores):
                    for cc in range(nchunks):
                        n0 = blk * shp + cc * ch
                        if li == 0:
                            xc = work.tile([cin, ch], BF16, tag="rcb")
                            nc.sync.dma_start(xc[:], xT[:, n0 : n0 + ch])
                            halves = [(xc, W1b, None)]
                        elif li == 1:
                            hc = work.tile([CO, ch], BF16, tag="rcb2")
                            nc.sync.dma_start(
                                hc[:],
                                hstackT[0][
                                    blk * CO : (blk + 1) * CO, cc * ch : (cc + 1) * ch
                                ],
                            )
                            sc = work.tile([CO, ch], BF16, tag="scb")
                            nc.sync.dma_start(sc[:], skipT[:, n0 : n0 + ch])
                            halves = [(hc, W1b, slice(0, CO)), (sc, W1bh, None)]
                        else:
                            hc = work.tile([CO, ch], BF16, tag="rcb2")
                            nc.sync.dma_start(
                                hc[:],
                                hstackT[1][
                                    blk * CO : (blk + 1) * CO, cc * ch : (cc + 1) * ch
                                ],
                            )
                            halves = [(hc, W2b, None)]
                        for ti in range(ntch):
                            ps = psum.tile([WIN, ROWP], F32, tag="pstab")
                            sl = slice(ti * WIN, (ti + 1) * WIN)
                            for c0, c1 in ((0, 512), (512, ROWP)):
                                for hi, (lh, wb, rows) in enumerate(halves):
                                    rhs = (
                                        wb[rows, c0:c1]
                                        if rows is not None
                                        else wb[:, c0:c1]
                                    )
                                    nc.tensor.matmul(
                                        ps[:, c0:c1],
                                        lhsT=lh[:, sl],
                                        rhs=rhs,
                                        start=(hi == 0),
                                        stop=(hi == len(halves) - 1),
                                    )
                            tb = work.tile([WIN, ROWP], BF16, tag="tbb")
                            nc.scalar.activation(tb[:], ps[:], AF.Copy)
                            nc.sync.dma_start(
                                table[n0 + ti * WIN : n0 + (ti + 1) * WIN, :], tb[:]
                            )

            def edge_phase(li):
                table = tables[li]
                rootb = root1b if li < 2 else root2b
                rw = ch // WIN
                GBM = gbmax
                rootc = None
                skownc = None
                rootc_cc = -1
                for v in range(nv):
                    # per-window PSUM accumulates across all 4 source groups,
                    # then the root-weight matmul joins the same accumulation
                    # group; ReLU reads PSUM directly (no SBUF hacc slab).
                    psw = psumw.tile([WIN, 576], F32, tag="psw")
                    for g in range(ngrp):
                        ntv = int(tcv[g][v])
                        t0 = int(voff[g][v])
                        bt = ntv
                        # bucket-aligned gather: trailing pad slots carry idx
                        # -1 and are trimmed by SWDGE desc-gen (no descriptor,
                        # no transfer); their stale SBUF slots are zeroed by
                        # the beta=0 multiply downstream.
                        g_tile = gpool.tile([WIN, GBM * ROWP], BF16, tag="g")
                        nc.gpsimd.dma_gather(
                            out_ap=g_tile[:, : bt * ROWP].rearrange(
                                "p (t c) -> p t c", c=ROWP
                            ),
                            in_ap=table[g * grows : (g + 1) * grows, :],
                            idxs_ap=src_sl[:, t0 * 8 : (t0 + bt) * 8],
                            num_idxs=bt * WIN,
                            num_idxs_reg=bt * WIN,
                            elem_size=ROWP,
                            single_packet=False,
                        )
                        # beta-apply: s-major rows, beta shipped as duplicated
                        # pairs so the o-broadcast rides an innermost packed
                        # pair (2x mode; one instr per tile to stay within 3
                        # free AP dims); then an all-contiguous add-tree
                        # reduces the 9 slots (2x mode at every level).
                        q = dpool.tile([WIN, GBM * ROWW], BF16, tag="q")
                        for ti in range(bt):
                            qv = (
                                g_tile[:, ti * ROWP : ti * ROWP + ROWW]
                                .rearrange(
                                    "p (s o k) -> p s o k", s=S, o=CO // 2, k=2
                                )
                            )
                            bv = (
                                betas[:, (t0 + ti) * 2 * S : (t0 + ti + 1) * 2 * S]
                                .rearrange("p (s x k) -> p s x k", x=1, k=2)
                                .to_broadcast([WIN, S, CO // 2, 2])
                            )
                            qq = q[
                                :, ti * ROWW : (ti + 1) * ROWW
                            ].rearrange("p (s o k) -> p s o k", s=S, o=CO // 2, k=2)
                            nc.vector.tensor_tensor(qq, qv, bv, op=OP.mult)
                        qs = q[:, : bt * ROWW].rearrange(
                            "p (t s o) -> p t s o", s=S, o=CO
                        )
                        r1 = dpool.tile([WIN, GBM * CO * 4], BF16, tag="r1")
                        r1v = r1[:, : bt * CO * 4].rearrange(
                            "p (t k o) -> p t k o", k=4, o=CO
                        )
                        nc.vector.tensor_tensor(
                            r1v, qs[:, :, 0:4, :], qs[:, :, 4:8, :], op=OP.add
                        )
                        r2 = dpool.tile([WIN, GBM * CO * 2], BF16, tag="r2")
                        r2v = r2[:, : bt * CO * 2].rearrange(
                            "p (t k o) -> p t k o", k=2, o=CO
                        )
                        nc.vector.tensor_tensor(
                            r2v, r1v[:, :, 0:2, :], r1v[:, :, 2:4, :], op=OP.add
                        )
                        d_acc = dpool.tile([WIN, GBM * CO], BF16, tag="dacc")
                        dav = d_acc[:, : bt * CO].rearrange(
                            "p (t o) -> p t o", o=CO
                        )
                        nc.vector.tensor_tensor(
                            dav, r2v[:, :, 0, :], r2v[:, :, 1, :], op=OP.add
                        )
                        nc.vector.tensor_tensor(
                            dav, dav, qs[:, :, 8, :], op=OP.add
                        )
                        # selection matrices for the whole bucket in one
                        # 2x-mode tensor_tensor (dst ids shipped as pairs)
                        d_sel = spool.tile([WIN, GBM * 256], BF16, tag="sel")
                        dsv = d_sel[:, : bt * 256].rearrange(
                            "p (t d k) -> p t d k", d=WIN, k=2
                        )
                        i0 = (
                            iota_t[:]
                            .rearrange("p (x d k) -> p x d k", x=1, k=2)
                            .to_broadcast([WIN, bt, WIN, 2])
                        )
                        i1 = (
                            dst_sl[:, t0 * 2 : (t0 + bt) * 2]
                            .rearrange("p (t x k) -> p t x k", x=1, k=2)
                            .to_broadcast([WIN, bt, WIN, 2])
                        )
                        nc.vector.tensor_tensor(dsv, i0, i1, op=OP.is_equal)
                        for j in range(ntv):
                            first = g == 0 and j == 0
                            nc.tensor.matmul(
                                psw[:, 0:CO],
                                lhsT=d_sel[:, j * 256 : j * 256 + WIN],
                                rhs=d_acc[:, j * CO : (j + 1) * CO],
                                start=first,
                                stop=False,
                            )
                            nc.tensor.matmul(
                                psw[:, 512 : 512 + CO],
                                lhsT=d_sel[:, j * 256 + WIN : (j + 1) * 256],
                                rhs=d_acc[:, j * CO : (j + 1) * CO],
                                start=first,
                                stop=False,
                            )
                    # root-weight contribution joins the PSUM accumulation
                    for half in range(2):
                        w = 2 * v + half
                        cc = w // rw
                        if cc != rootc_cc:
                            rootc_cc = cc
                            if li == 0:
                                rootc = work.tile([cin, ch], BF16, tag="rcb")
                                nc.sync.dma_start(
                                    rootc[:], xTown[:, cc * ch : (cc + 1) * ch]
                                )
                            else:
                                rootc = work.tile([CO, ch], BF16, tag="rcb2")
                                nc.sync.dma_start(
                                    rootc[:],
                                    bounceT[li - 1][:, cc * ch : (cc + 1) * ch],
                                )
                                if li == 1:
                                    skownc = work.tile([CO, ch], BF16, tag="rcb")
                                    nc.sync.dma_start(
                                        skownc[:],
                                        skipTown[:, cc * ch : (cc + 1) * ch],
                                    )
                        wsl = slice((w % rw) * WIN, (w % rw + 1) * WIN)
                        hsl = slice(half * 512, half * 512 + CO)
                        if li == 1:
                            nc.tensor.matmul(
                                psw[:, hsl], lhsT=rootc[:, wsl], rhs=rootb[0:CO, :],
                                start=False, stop=False,
                            )
                            nc.tensor.matmul(
                                psw[:, hsl], lhsT=skownc[:, wsl], rhs=root1bh[:, :],
                                start=False, stop=True,
                            )
                        else:
                            nc.tensor.matmul(
                                psw[:, hsl], lhsT=rootc[:, wsl], rhs=rootb[:, :],
                                start=False, stop=True,
                            )
                        # final: relu from PSUM + out/transpose
                        if li < 2:
                            hb = work.tile([WIN, CO], BF16, tag="hb")
                            nc.scalar.activation(hb[:], psw[:, hsl], AF.Relu)
                            pst = psum1.tile([CO, WIN], BF16, tag="ps2")
                            nc.tensor.transpose(
                                out=pst[:], in_=hb[:], identity=ident[:]
                            )
                            hbT = work.tile([CO, WIN], BF16, tag="hbT")
                            nc.scalar.activation(hbT[:], pst[:], AF.Copy)
                            nc.sync.dma_start(
                                bounceT[li][:, w * WIN : (w + 1) * WIN], hbT[:]
                            )
                        else:
                            ho = work.tile([WIN, CO], F32, tag="ho")
                            nc.scalar.activation(ho[:], psw[:, hsl], AF.Relu)
                            nc.sync.dma_start(
                                out_d[w * WIN : (w + 1) * WIN, :], ho[:]
                            )

            for li in range(3):
                table_phase(li)
                tc.strict_bb_all_engine_barrier()
                edge_phase(li)
                tc.strict_bb_all_engine_barrier()
                if li < 2:
                    nc.gpsimd.collective_compute(
                        "AllGather",
                        OP.bypass,
                        replica_groups=[list(range(cfg.ncores))],
                        ins=[bounceT[li].ap().opt()],
                        outs=[hstackT[li].ap().opt()],
                    )
                    tc.strict_bb_all_engine_barrier()
    nc.finalize()
    return nc


# --------------------------------------------------------------------------
# entry point
# --------------------------------------------------------------------------
def run_full(inputs, trace=False, trace_kwargs=None):
    cfg = FULL
    in_maps, sched = host_prep(
        cfg,
        np.asarray(inputs["x"], np.float32),
        np.asarray(inputs["skip"], np.float32),
        inputs["edge_index"],
        np.asarray(inputs["edge_attr"], np.float32),
        inputs["W1"],
        inputs["root1"],
        inputs["W2"],
        inputs["root2"],
    )
    nc = build_program(cfg, sched)
    from concourse.bass_utils import run_bass_kernel_spmd

    res = run_bass_kernel_spmd(
        nc,
        in_maps,
        core_ids=list(range(cfg.ncores)),
        trace=trace,
        **(dict(trace_kwargs=trace_kwargs) if trace_kwargs else {}),
    )
    out = np.zeros((cfg.n, CO), np.float32)
    for m in range(cfg.ncores):
        shard = res.results[m]["out_shard"]
        out[m * cfg.nsh : (m + 1) * cfg.nsh] = shard[: cfg.nsh]
    return out, res


def kernel(**inputs):
    out, _ = run_full(inputs)
    return out
